# revision 95
# baseline (speedup 1.0000x reference)
"""Trainium2 Bass kernel for batched per-sample expert matmul (MoE routing).

Computes y[n, i] = relu(b[idxs[n], i] + sum_o w[idxs[n], i, o] * x[n, o])
for x (8192, 256), idxs (8192,), w (64, 256, 256), b (64, 256).

Strategy (v2: expert-aligned shard, weight-stationary dataflow)
---------------------------------------------------------------
Host side (numpy, cheap):
  * Cut the batch into per-expert "units" of <= PAD samples (for the
    nominal input every expert has ~128 +- 15 samples, so unit == expert
    and PAD = 160).  Deal 8 units to each of the 8 cores; every unit has
    the same padded cost, so the load is perfectly balanced.
  * Per core, pre-transpose the unit samples so the contraction dim is
    on partitions (xt[p, c, j] = x_j[c*128 + p]) and slice the weight
    table into PE-stationary chunks (wstat[p, u, oc, ic, m] =
    w[e_u, ic*128+m, oc*128+p]).  All streams are fp16: precision is
    ample (values are O(1), accumulation stays fp32 in PSUM) and DMA
    bytes halve vs fp32.

Device side (one static Tile program, identical on all 8 cores —
per-core behaviour lives entirely in the DMA'd data):
  for each unit u, each output-feature chunk ic (128 features):
      psum[i, j]  = sum_p w[p,u,0,ic,i] * x0[p, j]   (K-chunk 0)
      psum[i, j] += sum_p w[p,u,1,ic,i] * x1[p, j]   (K-chunk 1)
      y[i, j] = relu(psum[i, j] + bias[i])   (drain, alternating DVE/ACT)

  With features on PSUM partitions the bias is a per-partition scalar,
  so it rides the drain op for free — no bias matmuls at all — and the
  weights are the stationary operand, so each expert's weights cross
  HBM exactly once (1 MB/core, a static slice of the table).

  DMA (the dominant cost — this kernel is HBM/descriptor bound):
  * Every [128, N] DMA costs one descriptor per partition at a few
    hundred ns each on its queue, so data is moved in FEW large
    transfers with fully-contiguous per-partition runs: the fp16 bias
    is packed into the head of the w stream, and x is laid out per-unit
    [c0|c1] so each batch is a single run.
  * The in-stream is spread over all three DGE issue rings (sync HWDGE,
    scalar HWDGE, gpsimd SWDGE) streaming concurrently — one ring alone
    sustains only a fraction of the ~360 GB/s HBM-per-core limit — with
    the head-critical w0 and x0 first on different rings.
  * y goes out in per-pair batches on rotating rings so the output
    stream overlaps the compute tail.
  * Two dummy matmuls absorb the w0/x0 semaphore waits; later batch
    boundaries are staggered so every first-consumer matmul carries at
    most one new wait (same-ring FIFO delivery implies the rest).

  PSUM: one private bank per unit, allocated up front — no pool-release
  waits, and no false cross-unit serialization from the Tile
  scheduler's per-tile dependency tracking (same reason y uses one SBUF
  tile per output batch).  A chain of wide dummy matmuls on a memset
  tile warms the tensor engine's p-state while the first batches are
  still in flight.

Host side: scatter unit rows back to the original sample order.
"""

import os

import numpy as np

import concourse.bacc as bacc
import concourse.bass as bass
import concourse.mybir as mybir
import concourse.tile as tile
from concourse.bass_utils import run_bass_kernel_spmd

N_CORES = 8
P = 128          # SBUF/PSUM partitions
F = 256          # feature dim (in_features == out_features == 256)
NE = 8           # units (experts) per core in the nominal case

MM_DT = mybir.dt.float16
NP_DT = np.float16
Y_DT = mybir.dt.float16

# Set by the last kernel() call when KBENCH_TRACE=1 (used by test.py only).
LAST_EXEC_TIME_NS = None
LAST_TRACE = None


def _build_units(idxs: np.ndarray, pad_cap: int = 512):
    """Cut the batch into single-expert units of <= pad_cap samples, sort by
    size, and deal them so slot j holds similar-sized units on every core.
    Returns (order, slot_pads, NE_eff, per_core): per_core[c][j] is
    (expert, start_in_order, count) and slot_pads[j] >= count for all cores."""
    order = np.argsort(idxs, kind="stable")
    sidx = idxs[order]
    # run-length encode the sorted expert ids
    bounds = np.flatnonzero(np.r_[True, sidx[1:] != sidx[:-1], True])
    units = []
    for lo, hi in zip(bounds[:-1], bounds[1:]):
        e = int(sidx[lo])
        k = int(lo)
        while k < hi:
            cnt = min(pad_cap, int(hi) - k)
            units.append((e, k, cnt))
            k += cnt
    units.sort(key=lambda u: -u[2])
    n_units = len(units)
    ne = (n_units + N_CORES - 1) // N_CORES
    units += [(0, 0, 0)] * (ne * N_CORES - n_units)
    per_core = [
        [units[j * N_CORES + c] for j in range(ne)] for c in range(N_CORES)
    ]
    slot_pads = [
        max(16, ((max(units[j * N_CORES + c][2] for c in range(N_CORES)) + 15)
                 // 16) * 16)
        for j in range(ne)
    ]
    return order, slot_pads, ne, per_core


def _batches(n, sizes, rest):
    out, lo, i = [], 0, 0
    while lo < n:
        sz = sizes[i] if i < len(sizes) else rest
        i += 1
        hi = min(n, lo + sz)
        out.append((lo, hi))
        lo = hi
    return out


def _build_program(ne: int, pads: list[int]):
    nc = bacc.Bacc(
        "TRN2", target_bir_lowering=False, debug=False, num_devices=N_CORES
    )
    ntot = sum(pads)
    xoff = [0]
    for p_ in pads:
        xoff.append(xoff[-1] + p_)
    # x folded per unit: cols [2*xoff[u], +2*pads[u]) = [c0 block | c1 block],
    # so each batch is one contiguous per-partition run = 1 DMA descriptor
    # per partition (descriptor count, not bytes, is what DMA time costs)
    xt_d = nc.dram_tensor("xt", [P, 2 * ntot], MM_DT, kind="ExternalInput").ap()
    # flat weights with the (fp16) bias packed in the first 16 columns, so the
    # bias rides the head weight batch instead of its own 64-byte-descriptor
    # DMA: cols [0,16) = bias[p, u*2+ic], then 512 cols per unit (oc, ic, m)
    w_d = nc.dram_tensor(
        "wstat", [P, 2 * ne + ne * 4 * P], MM_DT, kind="ExternalInput"
    ).ap()
    y_d = nc.dram_tensor("y", [P, 2 * ntot], Y_DT, kind="ExternalOutput").ap()

    f32 = mybir.dt.float32
    relu = mybir.ActivationFunctionType.Relu
    add = mybir.AluOpType.add
    amax = mybir.AluOpType.max

    # batch plans; stagger so no unit's first matmul sees a new w-batch AND a
    # new x-batch at once (unit 0's two waits are absorbed by dummy matmuls)
    wbat = _batches(ne, [1, 4], ne)          # w: [0],[1..4],[5..7]
    xbat = _batches(ne, [4], ne)             # x: [0..3],[4..7]
    obat = _batches(ne, [2, 2, 3], 1)        # y out: rotating rings, tiny final

    u2w = {}
    for g, (lo, hi) in enumerate(wbat):
        for u in range(lo, hi):
            u2w[u] = g
    u2x = {}
    for bx, (lo, hi) in enumerate(xbat):
        for u in range(lo, hi):
            u2x[u] = bx

    with tile.TileContext(nc) as tc:
        with (
            tc.tile_pool(name="const", bufs=1) as const,
            tc.tile_pool(name="w", bufs=1) as wpool,
            tc.tile_pool(name="x", bufs=1) as xpool,
            tc.tile_pool(name="yout", bufs=1) as ypool,
            tc.tile_pool(name="ps", bufs=1, space="PSUM") as pspool,
        ):
            xts = {}

            def load_x_batch(b, eng):
                lo, hi = xbat[b]
                span = xoff[hi] - xoff[lo]
                t = xpool.tile([P, 2 * span], MM_DT, tag=f"x{b}")
                xts[b] = t
                eng.dma_start(t[:], xt_d[:, 2 * xoff[lo]:2 * xoff[hi]])

            wts = {}

            def load_w_batch(g, eng):
                lo, hi = wbat[g]
                head = 2 * ne if g == 0 else 0
                t = wpool.tile(
                    [P, head + (hi - lo) * 4 * P], MM_DT, tag=f"w{g}"
                )
                wts[g] = t
                eng.dma_start(
                    t[:],
                    w_d[:, 2 * ne + lo * 4 * P - head:2 * ne + hi * 4 * P],
                )

            def xchunk(u, c):
                b = u2x[u]
                lo, _ = xbat[b]
                base = 2 * (xoff[u] - xoff[lo]) + c * pads[u]
                return xts[b][:, base:base + pads[u]]

            def wchunk(u, oc, ic):
                g = u2w[u]
                lo, _ = wbat[g]
                head = 2 * ne if g == 0 else 0
                base = head + ((u - lo) * 4 + oc * 2 + ic) * P
                return wts[g][:, base:base + P]

            # One DGE queue only sustains a fraction of the ~360 GB/s
            # HBM-per-core limit and small per-partition descriptors are
            # slower still, so the in-stream is spread over all three issue
            # rings (sync HWDGE, scalar HWDGE, gpsimd SWDGE) streaming
            # concurrently, in few large-descriptor batches.  Head batches
            # (w0+bias, x0) go first on their rings; same-ring FIFO delivery
            # keeps every later consumer to at most one new semaphore wait.
            # head-critical w0 and x0 ride DIFFERENT rings so both land at
            # the earliest possible time; second-wave batches queue behind,
            # one per ring, so no ring carries two large in-batches
            load_w_batch(0, nc.sync)     # bias + unit 0
            load_x_batch(0, nc.scalar)   # units 0-3
            if len(wbat) > 1:
                load_w_batch(1, nc.gpsimd)   # units 1-4
            if len(wbat) > 2:
                load_w_batch(2, nc.sync)     # units 5-7
            for g in range(3, len(wbat)):    # skew fallback only
                load_w_batch(g, nc.gpsimd)
            for b in range(1, len(xbat)):
                load_x_batch(b, nc.scalar)   # units 4-7
            bt = wts[0]                  # bias lives in w0's first 16 cols

            # ACT activation-table warm-up: pay the ~1.3us preamble while the
            # first DMA batches are still in flight.
            warm = const.tile([1, 2], f32, tag="warm")
            nc.vector.memset(warm[:], 0.0)
            nc.scalar.activation(warm[:], warm[:], relu)

            # PE p-state ramp: the tensor engine only reaches full clock
            # after ~3us of continuous work, so burn wide dummy matmuls on a
            # locally-initialized tile while the first DMA batches are still
            # in flight — the real stream then starts already warm.
            ramp = const.tile([P, 512], MM_DT, tag="ramp")
            nc.vector.memset(ramp[:], 0.0)

            # widen the packed fp16 bias to the fp32 per-partition scalars the
            # DVE/ACT drain ops require (one tiny ACT op once w0 lands)
            btf = const.tile([P, 2 * ne], f32, tag="btf")
            nc.scalar.copy(btf[:], bt[:, 0:2 * ne])

            # One PSUM bank per unit (both ic slots side by side), allocated
            # once up front: no pool-release waits, and — because the Tile
            # scheduler tracks dependencies per tile — no false serialization
            # of a later unit's matmuls behind an earlier unit's drains.
            nbank = min(8, ne)
            banks = [
                pspool.tile([P, 512], f32, name=f"pb{i}", tag=f"pb{i}")
                for i in range(nbank)
            ]

            if max(pads) <= 256:
                def psum_slot(s, width):
                    u, ic = s // 2, s % 2
                    return banks[u % nbank][:, ic * width:(ic + 1) * width]
            else:
                def psum_slot(s, width):
                    return banks[s % nbank][:, 0:width]

            scr = banks[0][0:2, :]
            # one y tile per output batch, so an output DMA's read never
            # blocks a later unit's drain via tile-granular dependencies
            yts = {}
            for oi, (lo, hi) in enumerate(obat):
                yts[oi] = ypool.tile(
                    [P, 2 * (xoff[hi] - xoff[lo])], Y_DT,
                    name=f"y{oi}", tag=f"y{oi}",
                )

            def ytile(u):
                for oi, (lo, hi) in enumerate(obat):
                    if lo <= u < hi:
                        return yts[oi], 2 * xoff[lo], oi
                raise AssertionError

            for _ in range(9):
                nc.tensor.matmul(
                    scr, ramp[:, 0:2], ramp[:], start=True, stop=True
                )

            for u in range(ne):
                if u == 0:
                    # absorb the w-batch-0 and x-batch-0 semaphore waits so
                    # unit 0's real matmuls carry none
                    wb = wts[0]
                    nc.tensor.matmul(
                        scr[:, 0:2], wb[:, 0:2], wb[:, 0:2],
                        start=True, stop=True,
                    )
                    xb = xts[0]
                    nc.tensor.matmul(
                        scr[:, 0:2], xb[:, 0:2], xb[:, 0:2],
                        start=True, stop=True,
                    )
                if u == 1:
                    # fill the wait for the w1 batch with ramp matmuls so the
                    # PE's activity monitor stays hot into the dense stream;
                    # they write the LAST unit's bank (no reader until its
                    # drains, so no cross-engine wait gets inserted)
                    for _ in range(3):
                        nc.tensor.matmul(
                            banks[nbank - 1][0:2, :], ramp[:, 0:2], ramp[:],
                            start=True, stop=True,
                        )
                if u == 5:
                    # same for the shorter w2/x1 wait
                    nc.tensor.matmul(
                        banks[nbank - 1][0:2, :], ramp[:, 0:2], ramp[:],
                        start=True, stop=True,
                    )

                for ic in range(2):
                    pd = pads[u]
                    ps = psum_slot(u * 2 + ic, pd)
                    nc.tensor.matmul(
                        ps, wchunk(u, 0, ic), xchunk(u, 0),
                        start=True, stop=False,
                    )
                    nc.tensor.matmul(
                        ps, wchunk(u, 1, ic), xchunk(u, 1),
                        start=False, stop=True,
                    )
                    yt, ybase, oi = ytile(u)
                    o = 2 * xoff[u] - ybase + ic * pd
                    bv = btf[:, u * 2 + ic:u * 2 + ic + 1]
                    if ic == 0:
                        nc.vector.tensor_scalar(
                            yt[:, o:o + pd], ps, bv, 0.0, add, amax
                        )
                    else:
                        nc.scalar.activation(
                            yt[:, o:o + pd], ps, relu, bias=bv
                        )
                for oi, (lo, hi) in enumerate(obat):
                    if u == hi - 1:
                        # per-batch outputs on rotating rings with a tiny
                        # final batch: the output stream overlaps the compute
                        # tail and the last transfer is short
                        eng = (nc.sync, nc.gpsimd, nc.scalar, nc.scalar)[oi % 4]
                        eng.dma_start(
                            y_d[:, 2 * xoff[lo]:2 * xoff[hi]],
                            yts[oi][:],
                        )
    nc.compile()
    return nc


def kernel(x: np.ndarray, idxs: np.ndarray, w: np.ndarray, b: np.ndarray) -> np.ndarray:
    global LAST_EXEC_TIME_NS, LAST_TRACE
    x = np.ascontiguousarray(x, dtype=np.float32)
    w = np.ascontiguousarray(w, dtype=np.float32)
    b = np.ascontiguousarray(b, dtype=np.float32)
    idxs_np = np.asarray(idxs).astype(np.int64)

    B, Fdim = x.shape
    order, pads, ne, per_core = _build_units(idxs_np)
    ntot = sum(pads)
    xoff = np.concatenate([[0], np.cumsum(pads)]).astype(np.int64)

    # wprep[p, e, oc, ic, m] = w[e, ic*128+m, oc*128+p]
    wprep = np.ascontiguousarray(
        w.reshape(64, 2, P, 2, P).transpose(4, 0, 3, 1, 2).astype(NP_DT)
    )
    # bprep[p, e, ic] = b[e, ic*128+p]
    bprep = np.ascontiguousarray(b.reshape(64, 2, P).transpose(2, 0, 1))

    nc = _build_program(ne, pads)
    trace = bool(os.environ.get("KBENCH_TRACE"))

    in_maps = []
    for c in range(N_CORES):
        units = per_core[c]
        eids = np.array([u[0] for u in units])
        # xt[p, 2*xoff[u] + c*pads[u] + j] = x_sample(u,j)[c*128 + p]
        xt = np.zeros((P, 2 * ntot), dtype=NP_DT)
        for s, (e, k0, cnt) in enumerate(units):
            pd = pads[s]
            blk = np.zeros((pd, 2, P), dtype=np.float32)
            blk[:cnt] = x[order[k0:k0 + cnt]].reshape(cnt, 2, P)
            xt[:, 2 * xoff[s]:2 * xoff[s] + 2 * pd] = (
                blk.transpose(2, 1, 0).reshape(P, 2 * pd).astype(NP_DT)
            )
        wstat = np.ascontiguousarray(np.concatenate(
            [
                bprep[:, eids].reshape(P, 2 * ne).astype(NP_DT),
                wprep[:, eids].reshape(P, ne * 4 * P),
            ],
            axis=1,
        ))
        in_maps.append({"xt": xt, "wstat": wstat})

    res = run_bass_kernel_spmd(
        nc, in_maps, core_ids=list(range(N_CORES)), trace=trace
    )
    LAST_EXEC_TIME_NS = res.exec_time_ns
    LAST_TRACE = res.instructions_and_trace

    y = np.empty((B, Fdim), dtype=np.float32)
    for c in range(N_CORES):
        units = per_core[c]
        yc = res.results[c]["y"].astype(np.float32)  # [128, 2*ntot]
        for s, (e, k0, cnt) in enumerate(units):
            if cnt == 0:
                continue
            pd = pads[s]
            blk = yc[:, 2 * xoff[s]:2 * xoff[s] + 2 * pd]
            # blk[m, ic*pd + j] -> sample j, feature ic*128+m
            y[order[k0:k0 + cnt]] = (
                blk.reshape(P, 2, pd).transpose(2, 1, 0).reshape(pd, Fdim)[:cnt]
            )
    return y


# revision 97
# speedup vs baseline: 1.1103x; 1.1103x over previous
"""Trainium2 Bass kernel for batched per-sample expert matmul (MoE routing).

Computes y[n, i] = relu(b[idxs[n], i] + sum_o w[idxs[n], i, o] * x[n, o])
for x (8192, 256), idxs (8192,), w (64, 256, 256), b (64, 256).

Strategy (v2: expert-aligned shard, weight-stationary dataflow)
---------------------------------------------------------------
Host side (numpy, cheap):
  * Cut the batch into per-expert "units" of <= PAD samples (for the
    nominal input every expert has ~128 +- 15 samples, so unit == expert
    and PAD = 160).  Deal 8 units to each of the 8 cores; every unit has
    the same padded cost, so the load is perfectly balanced.
  * Per core, pre-transpose the unit samples so the contraction dim is
    on partitions (xt[p, c, j] = x_j[c*128 + p]) and slice the weight
    table into PE-stationary chunks (wstat[p, u, oc, ic, m] =
    w[e_u, ic*128+m, oc*128+p]).  All streams are fp16: precision is
    ample (values are O(1), accumulation stays fp32 in PSUM) and DMA
    bytes halve vs fp32.

Device side (one static Tile program, identical on all 8 cores —
per-core behaviour lives entirely in the DMA'd data):
  for each unit u, each output-feature chunk ic (128 features):
      psum[i, j]  = sum_p w[p,u,0,ic,i] * x0[p, j]   (K-chunk 0)
      psum[i, j] += sum_p w[p,u,1,ic,i] * x1[p, j]   (K-chunk 1)
      y[i, j] = relu(psum[i, j] + bias[i])   (drain, alternating DVE/ACT)

  With features on PSUM partitions the bias is a per-partition scalar,
  so it rides the drain op for free — no bias matmuls at all — and the
  weights are the stationary operand, so each expert's weights cross
  HBM exactly once (1 MB/core, a static slice of the table).

  DMA (the dominant cost — this kernel is HBM/descriptor bound):
  * Every [128, N] DMA costs one descriptor per partition at a few
    hundred ns each on its queue, so data is moved in FEW large
    transfers with fully-contiguous per-partition runs: the fp16 bias
    is packed into the head of the w stream, and x is laid out per-unit
    [c0|c1] so each batch is a single run.
  * The in-stream is spread over all three DGE issue rings (sync HWDGE,
    scalar HWDGE, gpsimd SWDGE) streaming concurrently — one ring alone
    sustains only a fraction of the ~360 GB/s HBM-per-core limit — with
    the head-critical w0 and x0 first on different rings.
  * y goes out in per-pair batches on rotating rings so the output
    stream overlaps the compute tail.
  * Two dummy matmuls absorb the w0/x0 semaphore waits; later batch
    boundaries are staggered so every first-consumer matmul carries at
    most one new wait (same-ring FIFO delivery implies the rest).

  PSUM: one private bank per unit, allocated up front — no pool-release
  waits, and no false cross-unit serialization from the Tile
  scheduler's per-tile dependency tracking (same reason y uses one SBUF
  tile per output batch).  A chain of wide dummy matmuls on a memset
  tile warms the tensor engine's p-state while the first batches are
  still in flight.

Host side: scatter unit rows back to the original sample order.
"""

import os

import numpy as np

import concourse.bacc as bacc
import concourse.bass as bass
import concourse.mybir as mybir
import concourse.tile as tile
from concourse.bass_utils import run_bass_kernel_spmd

N_CORES = 8
P = 128          # SBUF/PSUM partitions
F = 256          # feature dim (in_features == out_features == 256)
NE = 8           # units (experts) per core in the nominal case

MM_DT = mybir.dt.float16
NP_DT = np.float16
Y_DT = mybir.dt.float16

# Set by the last kernel() call when KBENCH_TRACE=1 (used by test.py only).
LAST_EXEC_TIME_NS = None
LAST_TRACE = None


def _build_units(idxs: np.ndarray, pad_cap: int = 512):
    """Cut the batch into single-expert units of <= pad_cap samples, sort by
    size, and deal them so slot j holds similar-sized units on every core.
    Returns (order, slot_pads, NE_eff, per_core): per_core[c][j] is
    (expert, start_in_order, count) and slot_pads[j] >= count for all cores."""
    order = np.argsort(idxs, kind="stable")
    sidx = idxs[order]
    # run-length encode the sorted expert ids
    bounds = np.flatnonzero(np.r_[True, sidx[1:] != sidx[:-1], True])
    units = []
    for lo, hi in zip(bounds[:-1], bounds[1:]):
        e = int(sidx[lo])
        k = int(lo)
        while k < hi:
            cnt = min(pad_cap, int(hi) - k)
            units.append((e, k, cnt))
            k += cnt
    units.sort(key=lambda u: -u[2])
    n_units = len(units)
    ne = (n_units + N_CORES - 1) // N_CORES
    units += [(0, 0, 0)] * (ne * N_CORES - n_units)
    per_core = [
        [units[j * N_CORES + c] for j in range(ne)] for c in range(N_CORES)
    ]
    slot_pads = [
        max(16, ((max(units[j * N_CORES + c][2] for c in range(N_CORES)) + 15)
                 // 16) * 16)
        for j in range(ne)
    ]
    return order, slot_pads, ne, per_core


def _batches(n, sizes, rest):
    out, lo, i = [], 0, 0
    while lo < n:
        sz = sizes[i] if i < len(sizes) else rest
        i += 1
        hi = min(n, lo + sz)
        out.append((lo, hi))
        lo = hi
    return out


def _build_program(ne: int, pads: list[int]):
    nc = bacc.Bacc(
        "TRN2", target_bir_lowering=False, debug=False, num_devices=N_CORES
    )
    ntot = sum(pads)
    xoff = [0]
    for p_ in pads:
        xoff.append(xoff[-1] + p_)
    # x folded per unit: cols [2*xoff[u], +2*pads[u]) = [c0 block | c1 block],
    # so each batch is one contiguous per-partition run = 1 DMA descriptor
    # per partition (descriptor count, not bytes, is what DMA time costs)
    xt_d = nc.dram_tensor("xt", [P, 2 * ntot], MM_DT, kind="ExternalInput").ap()
    # flat weights with the (fp16) bias packed in the first 16 columns, so the
    # bias rides the head weight batch instead of its own 64-byte-descriptor
    # DMA: cols [0,16) = bias[p, u*2+ic], then 512 cols per unit (oc, ic, m)
    w_d = nc.dram_tensor(
        "wstat", [P, 2 * ne + ne * 4 * P], MM_DT, kind="ExternalInput"
    ).ap()
    y_d = nc.dram_tensor("y", [P, 2 * ntot], Y_DT, kind="ExternalOutput").ap()

    f32 = mybir.dt.float32
    relu = mybir.ActivationFunctionType.Relu
    add = mybir.AluOpType.add
    amax = mybir.AluOpType.max

    # batch plans; stagger so no unit's first matmul sees a new w-batch AND a
    # new x-batch at once (unit 0's two waits are absorbed by dummy matmuls)
    wbat = _batches(ne, [1, 4], ne)          # w: [0],[1..4],[5..7]
    xbat = _batches(ne, [4], ne)             # x: [0..3],[4..7]
    obat = _batches(ne, [2, 2, 3], 1)        # y out: rotating rings, tiny final

    u2w = {}
    for g, (lo, hi) in enumerate(wbat):
        for u in range(lo, hi):
            u2w[u] = g
    u2x = {}
    for bx, (lo, hi) in enumerate(xbat):
        for u in range(lo, hi):
            u2x[u] = bx

    with tile.TileContext(nc) as tc:
        with (
            tc.tile_pool(name="const", bufs=1) as const,
            tc.tile_pool(name="w", bufs=1) as wpool,
            tc.tile_pool(name="x", bufs=1) as xpool,
            tc.tile_pool(name="yout", bufs=1) as ypool,
            tc.tile_pool(name="ps", bufs=1, space="PSUM") as pspool,
        ):
            xts = {}

            def load_x_batch(b, eng):
                lo, hi = xbat[b]
                span = xoff[hi] - xoff[lo]
                t = xpool.tile([P, 2 * span], MM_DT, tag=f"x{b}")
                xts[b] = t
                eng.dma_start(t[:], xt_d[:, 2 * xoff[lo]:2 * xoff[hi]])

            wts = {}

            def load_w_batch(g, eng):
                lo, hi = wbat[g]
                head = 2 * ne if g == 0 else 0
                t = wpool.tile(
                    [P, head + (hi - lo) * 4 * P], MM_DT, tag=f"w{g}"
                )
                wts[g] = t
                eng.dma_start(
                    t[:],
                    w_d[:, 2 * ne + lo * 4 * P - head:2 * ne + hi * 4 * P],
                )

            def xchunk(u, c):
                b = u2x[u]
                lo, _ = xbat[b]
                base = 2 * (xoff[u] - xoff[lo]) + c * pads[u]
                return xts[b][:, base:base + pads[u]]

            def wchunk(u, oc, ic):
                g = u2w[u]
                lo, _ = wbat[g]
                head = 2 * ne if g == 0 else 0
                base = head + ((u - lo) * 4 + oc * 2 + ic) * P
                return wts[g][:, base:base + P]

            # One DGE queue only sustains a fraction of the ~360 GB/s
            # HBM-per-core limit and small per-partition descriptors are
            # slower still, so the in-stream is spread over all three issue
            # rings (sync HWDGE, scalar HWDGE, gpsimd SWDGE) streaming
            # concurrently, in few large-descriptor batches.  Head batches
            # (w0+bias, x0) go first on their rings; same-ring FIFO delivery
            # keeps every later consumer to at most one new semaphore wait.
            # head-critical w0 and x0 ride DIFFERENT rings so both land at
            # the earliest possible time; second-wave batches queue behind,
            # one per ring, so no ring carries two large in-batches
            load_w_batch(0, nc.sync)     # bias + unit 0
            load_x_batch(0, nc.scalar)   # units 0-3
            if len(wbat) > 1:
                load_w_batch(1, nc.gpsimd)   # units 1-4
            if len(wbat) > 2:
                load_w_batch(2, nc.sync)     # units 5-7
            for g in range(3, len(wbat)):    # skew fallback only
                load_w_batch(g, nc.gpsimd)
            for b in range(1, len(xbat)):
                load_x_batch(b, nc.scalar)   # units 4-7
            bt = wts[0]                  # bias lives in w0's first 16 cols

            # ACT activation-table warm-up: pay the ~1.3us preamble while the
            # first DMA batches are still in flight.
            warm = const.tile([1, 2], f32, tag="warm")
            nc.vector.memset(warm[:], 0.0)
            nc.scalar.activation(warm[:], warm[:], relu)

            # PE p-state ramp: the tensor engine only reaches full clock
            # after ~3us of continuous work, so burn wide dummy matmuls on a
            # locally-initialized tile while the first DMA batches are still
            # in flight — the real stream then starts already warm.
            ramp = const.tile([P, 512], MM_DT, tag="ramp")
            nc.vector.memset(ramp[:], 0.0)

            # widen the packed fp16 bias to the fp32 per-partition scalars the
            # DVE/ACT drain ops require (one tiny ACT op once w0 lands)
            btf = const.tile([P, 2 * ne], f32, tag="btf")
            nc.scalar.copy(btf[:], bt[:, 0:2 * ne])

            # One PSUM bank per unit (both ic slots side by side), allocated
            # once up front: no pool-release waits, and — because the Tile
            # scheduler tracks dependencies per tile — no false serialization
            # of a later unit's matmuls behind an earlier unit's drains.
            nbank = min(8, ne)
            banks = [
                pspool.tile([P, 512], f32, name=f"pb{i}", tag=f"pb{i}")
                for i in range(nbank)
            ]

            if max(pads) <= 256:
                def psum_slot(s, width):
                    u, ic = s // 2, s % 2
                    return banks[u % nbank][:, ic * width:(ic + 1) * width]
            else:
                def psum_slot(s, width):
                    return banks[s % nbank][:, 0:width]

            scr = banks[0][0:2, :]
            # one y tile per output batch, so an output DMA's read never
            # blocks a later unit's drain via tile-granular dependencies
            yts = {}
            for oi, (lo, hi) in enumerate(obat):
                yts[oi] = ypool.tile(
                    [P, 2 * (xoff[hi] - xoff[lo])], Y_DT,
                    name=f"y{oi}", tag=f"y{oi}",
                )

            def ytile(u):
                for oi, (lo, hi) in enumerate(obat):
                    if lo <= u < hi:
                        return yts[oi], 2 * xoff[lo], oi
                raise AssertionError

            for _ in range(9):
                nc.tensor.matmul(
                    scr, ramp[:, 0:2], ramp[:], start=True, stop=True
                )

            for u in range(ne):
                if u == 0:
                    # absorb the w-batch-0 and x-batch-0 semaphore waits so
                    # unit 0's real matmuls carry none
                    wb = wts[0]
                    nc.tensor.matmul(
                        scr[:, 0:2], wb[:, 0:2], wb[:, 0:2],
                        start=True, stop=True,
                    )
                    xb = xts[0]
                    nc.tensor.matmul(
                        scr[:, 0:2], xb[:, 0:2], xb[:, 0:2],
                        start=True, stop=True,
                    )
                if u == 1:
                    # fill the wait for the w1 batch with ramp matmuls so the
                    # PE's activity monitor stays hot into the dense stream;
                    # they write the LAST unit's bank (no reader until its
                    # drains, so no cross-engine wait gets inserted)
                    for _ in range(5):
                        nc.tensor.matmul(
                            banks[nbank - 1][0:2, :], ramp[:, 0:2], ramp[:],
                            start=True, stop=True,
                        )


                for ic in range(2):
                    pd = pads[u]
                    ps = psum_slot(u * 2 + ic, pd)
                    nc.tensor.matmul(
                        ps, wchunk(u, 0, ic), xchunk(u, 0),
                        start=True, stop=False,
                    )
                    nc.tensor.matmul(
                        ps, wchunk(u, 1, ic), xchunk(u, 1),
                        start=False, stop=True,
                    )
                    yt, ybase, oi = ytile(u)
                    o = 2 * xoff[u] - ybase + ic * pd
                    bv = btf[:, u * 2 + ic:u * 2 + ic + 1]
                    if ic == 0:
                        nc.vector.tensor_scalar(
                            yt[:, o:o + pd], ps, bv, 0.0, add, amax
                        )
                    else:
                        nc.scalar.activation(
                            yt[:, o:o + pd], ps, relu, bias=bv
                        )
                for oi, (lo, hi) in enumerate(obat):
                    if u == hi - 1:
                        # per-batch outputs on rotating rings with a tiny
                        # final batch: the output stream overlaps the compute
                        # tail and the last transfer is short
                        eng = (nc.sync, nc.gpsimd, nc.scalar, nc.scalar)[oi % 4]
                        eng.dma_start(
                            y_d[:, 2 * xoff[lo]:2 * xoff[hi]],
                            yts[oi][:],
                        )
    nc.compile()
    return nc


def kernel(x: np.ndarray, idxs: np.ndarray, w: np.ndarray, b: np.ndarray) -> np.ndarray:
    global LAST_EXEC_TIME_NS, LAST_TRACE
    x = np.ascontiguousarray(x, dtype=np.float32)
    w = np.ascontiguousarray(w, dtype=np.float32)
    b = np.ascontiguousarray(b, dtype=np.float32)
    idxs_np = np.asarray(idxs).astype(np.int64)

    B, Fdim = x.shape
    order, pads, ne, per_core = _build_units(idxs_np)
    ntot = sum(pads)
    xoff = np.concatenate([[0], np.cumsum(pads)]).astype(np.int64)

    # wprep[p, e, oc, ic, m] = w[e, ic*128+m, oc*128+p]
    wprep = np.ascontiguousarray(
        w.reshape(64, 2, P, 2, P).transpose(4, 0, 3, 1, 2).astype(NP_DT)
    )
    # bprep[p, e, ic] = b[e, ic*128+p]
    bprep = np.ascontiguousarray(b.reshape(64, 2, P).transpose(2, 0, 1))

    nc = _build_program(ne, pads)
    trace = bool(os.environ.get("KBENCH_TRACE"))

    in_maps = []
    for c in range(N_CORES):
        units = per_core[c]
        eids = np.array([u[0] for u in units])
        # xt[p, 2*xoff[u] + c*pads[u] + j] = x_sample(u,j)[c*128 + p]
        xt = np.zeros((P, 2 * ntot), dtype=NP_DT)
        for s, (e, k0, cnt) in enumerate(units):
            pd = pads[s]
            blk = np.zeros((pd, 2, P), dtype=np.float32)
            blk[:cnt] = x[order[k0:k0 + cnt]].reshape(cnt, 2, P)
            xt[:, 2 * xoff[s]:2 * xoff[s] + 2 * pd] = (
                blk.transpose(2, 1, 0).reshape(P, 2 * pd).astype(NP_DT)
            )
        wstat = np.ascontiguousarray(np.concatenate(
            [
                bprep[:, eids].reshape(P, 2 * ne).astype(NP_DT),
                wprep[:, eids].reshape(P, ne * 4 * P),
            ],
            axis=1,
        ))
        in_maps.append({"xt": xt, "wstat": wstat})

    res = run_bass_kernel_spmd(
        nc, in_maps, core_ids=list(range(N_CORES)), trace=trace
    )
    LAST_EXEC_TIME_NS = res.exec_time_ns
    LAST_TRACE = res.instructions_and_trace

    y = np.empty((B, Fdim), dtype=np.float32)
    for c in range(N_CORES):
        units = per_core[c]
        yc = res.results[c]["y"].astype(np.float32)  # [128, 2*ntot]
        for s, (e, k0, cnt) in enumerate(units):
            if cnt == 0:
                continue
            pd = pads[s]
            blk = yc[:, 2 * xoff[s]:2 * xoff[s] + 2 * pd]
            # blk[m, ic*pd + j] -> sample j, feature ic*128+m
            y[order[k0:k0 + cnt]] = (
                blk.reshape(P, 2, pd).transpose(2, 1, 0).reshape(pd, Fdim)[:cnt]
            )
    return y
